# revision 24
# baseline (speedup 1.0000x reference)
"""Trainium2 Bass kernel for CustomWindowMHA (sparse window+dilated attention).

Problem (hardcoded):
  x: (2, 2048, 1024) f32, qkv: (3072, 1024) f32, wo: (1024, 1024) f32
  H=16 heads, dh=64, window=128, dilation=4.
  out = softmax(mask(QK^T/8)) V @ wo^T          (B, S, D) f32

Sharding: 16 heads / 8 cores = 2 heads per core (head-parallel).  Each core
computes its 2 heads' attention output O_d (n=4096, 128) and the partial
Y_d = O_d @ wo[:, d-slice]^T (4096, 1024).  Host sums the 8 partials.

Per-core layout (everything chained transposed so activations never need a
transpose; compute dtype bf16 with fp32 PSUM accumulation):
  xT (1024, 4096) bf16 [f, n]   host-pretransposed, replicated
  QT = (wq_d/8) @ xT : (128, 4096)  [2*dh, n]   (1/8 folded into weights)
  KT =  wk_d    @ xT : (128, 4096)
  VT =  wv_d    @ xT : (128, 4096) -> PE-transpose -> V natural (j, dh) bf16
  Attention is split into two streams that accumulate into one PSUM tile:
  - window/mixed stream (j-blocks near the diagonal): scores_T per head,
    exp (ACT), multiplicative 0/1 W01[delta] mask (DVE), AV with [V|1]
    aug so row 64 carries sum(exp); leading all-invalid columns trimmed.
  - deep-dilated stream (j < 128*(4c-2), where delta >= 257 everywhere):
    residue-compacted:  for r = i%4, S_r[jm, im] = K[4jm+r] . Q[4im+r];
    every computed entry is valid -> NO mask work at all.  AV uses
    stride-4-gathered V_r and writes strided PSUM columns i%4==r of the
    same O_aug.T accumulator.
  O.T = O_aug.T[0:64] * (1/row64)  (DVE recip + gpsimd partition_broadcast)
  Y tile = O2.T-slice^T . woT -> SBUF -> DMA   (f32 out, host sums partials)
"""

import os
import sys

import numpy as np

for _p in ("/opt/trn_rl_repo",):
    if _p not in sys.path and os.path.isdir(_p):
        sys.path.append(_p)

import ml_dtypes
import concourse.bacc as bacc
import concourse.bass as bass
import concourse.mybir as mybir
import concourse.tile as tile
from concourse.bass_utils import run_bass_kernel_spmd
from concourse.masks import make_identity

F32 = mybir.dt.float32
BF16 = mybir.dt.bfloat16
NPBF16 = ml_dtypes.bfloat16

B, S, D = 2, 2048, 1024
H, DH = 16, 64
WINDOW, DIL = 128, 4
NCORES = 8
N = B * S          # 4096 rows total
DH2 = 2 * DH       # 128 dims per core (2 heads)
NCH = N // 512     # 8 n-chunks of 512
CPB = S // 512     # 4 i-chunks per batch
JB = S // 128      # 16 j-blocks per batch
T0 = 384           # W01 band offset: delta = t - T0 - j'
TW = T0 + 512 * (CPB - 1) + 512  # 2432 band width
COMPACT = os.environ.get("K_COMPACT", "1") == "1"
TRIM = os.environ.get("K_TRIM", "1") == "1"


def build_tile_kernel(tc):
    nc = tc.nc

    xT = nc.dram_tensor("xT", [128, 8, N], BF16, kind="ExternalInput").ap()
    wqkvT = nc.dram_tensor("wqkvT", [128, 8, 3 * DH2], BF16, kind="ExternalInput").ap()
    woT = nc.dram_tensor("woT", [DH2, D], BF16, kind="ExternalInput").ap()
    w01 = nc.dram_tensor("w01", [128, TW], BF16, kind="ExternalInput").ap()
    y = nc.dram_tensor("y", [N, D], BF16, kind="ExternalOutput").ap()

    with (
        tc.tile_pool(name="const", bufs=1) as cpool,
        tc.tile_pool(name="xin", bufs=3) as xpool,
        tc.tile_pool(name="qk", bufs=8) as qkpool,
        tc.tile_pool(name="vtp", bufs=2) as vtpool,
        tc.tile_pool(name="pers", bufs=1) as perspool,
        tc.tile_pool(name="exp", bufs=4) as epool,
        tc.tile_pool(name="ot2", bufs=4) as opool,
        tc.tile_pool(name="small", bufs=4) as spool,
        tc.tile_pool(name="yst", bufs=2) as ypool,
    ):
        # ---- constants / weights in SBUF ----
        wqkv_sb = cpool.tile([128, 8, 3 * DH2], BF16, name="wqkv_sb")
        nc.sync.dma_start(wqkv_sb, wqkvT)
        wo_sb = cpool.tile([128, D], BF16, name="wo_sb")
        w01_sb = cpool.tile([128, TW], BF16, name="w01_sb")
        ident = cpool.tile([128, 128], BF16, name="ident")
        make_identity(nc, ident)

        # per-head V with ones column, interleaved: per j-block g a 130-col
        # group [V_h0(64) | 1 | V_h1(64) | 1]
        vhh = perspool.tile([128, 2 * JB * 130], BF16, name="vhh")
        nc.vector.memset(
            vhh.rearrange("p (g w) -> p g w", w=65)[:, :, 64:65], 1.0)
        # residue-gathered V for the compact stream: (b, r, jmb<3) 130-groups
        vrr = perspool.tile([128, B * 4 * 3 * 130], BF16, name="vrr")
        nc.vector.memset(
            vrr.rearrange("p (g w) -> p g w", w=65)[:, :, 64:65], 1.0)

        qts = [None] * NCH
        ktb = [None] * B
        vtb = [None] * B

        for bb in range(B):
            with (
                tc.tile_pool(name=f"pp1_{bb}", bufs=2, space="PSUM") as pp1,
                tc.tile_pool(name=f"psc_{bb}", bufs=3, space="PSUM") as psc,
                tc.tile_pool(name=f"pot_{bb}", bufs=3, space="PSUM") as pot,
            ):
                b = bb
                kt = qkpool.tile([128, S], BF16, tag="kt", name=f"ktb{bb}", bufs=2)
                vt = vtpool.tile([128, S], BF16, tag="vt", name=f"vtb{bb}", bufs=2)
                ktb[bb] = kt
                vtb[bb] = vt
                vt4 = vt.rearrange("p (j r) -> p r j", r=4)
                for cc in range(CPB):
                    # ---------- projections for chunk cc ----------
                    ci = bb * CPB + cc
                    n0 = ci * 512
                    xtb = xpool.tile([128, 8, 512], BF16, tag="xt", name=f"xt{ci}")
                    if ci == 0:
                        # first f-tile alone so the very first matmul can start
                        nc.sync.dma_start(xtb[:, 0, :], xT[:, 0, n0:n0 + 512])
                        nc.sync.dma_start(xtb[:, 1:8, :], xT[:, 1:8, n0:n0 + 512])
                        nc.sync.dma_start(wo_sb, woT)
                        nc.sync.dma_start(w01_sb, w01)
                    else:
                        nc.sync.dma_start(xtb, xT[:, :, n0:n0 + 512])
                    xts = [xtb[:, ft, :] for ft in range(8)]

                    psq = pp1.tile([128, 512], F32, tag="proj", name=f"psq{ci}")
                    for ft in range(8):
                        nc.tensor.matmul(psq, wqkv_sb[:, ft, 0:DH2], xts[ft],
                                         start=(ft == 0), stop=(ft == 7))
                    qt = qkpool.tile([128, 512], BF16, tag="qt", name=f"qt{ci}")
                    nc.scalar.copy(qt, psq)
                    qts[ci] = qt

                    psk = pp1.tile([128, 512], F32, tag="proj", name=f"psk{ci}")
                    for ft in range(8):
                        nc.tensor.matmul(psk, wqkv_sb[:, ft, DH2:2 * DH2], xts[ft],
                                         start=(ft == 0), stop=(ft == 7))
                    nc.scalar.copy(kt[:, cc * 512:cc * 512 + 512], psk)

                    psv = pp1.tile([128, 512], F32, tag="proj", name=f"psv{ci}")
                    for ft in range(8):
                        nc.tensor.matmul(psv, wqkv_sb[:, ft, 2 * DH2:3 * DH2], xts[ft],
                                         start=(ft == 0), stop=(ft == 7))
                    nc.scalar.copy(vt[:, cc * 512:cc * 512 + 512], psv)

                    # natural V blocks for this chunk
                    for sub in range(4):
                        g = ci * 4 + sub
                        pvt = pp1.tile([128, 128], BF16, tag="proj", name=f"pvt{g}")
                        nc.tensor.transpose(
                            pvt, vt[:, cc * 512 + sub * 128:cc * 512 + (sub + 1) * 128],
                            ident)
                        dst = vhh[:, g * 130:g * 130 + 130].rearrange(
                            "p (two w) -> p two w", two=2)[:, :, 0:64]
                        nc.vector.tensor_copy(
                            dst, pvt.rearrange("p (two w) -> p two w", two=2))
                    # residue-gathered V_r block cc (needed by chunks > cc)
                    if cc < 3:
                        for r in range(4):
                            pvr = pp1.tile([128, 128], BF16, tag="proj",
                                            name=f"pvr{bb}_{r}_{cc}")
                            nc.tensor.transpose(
                                pvr, vt4[:, r, cc * 128:(cc + 1) * 128], ident)
                            off = ((bb * 4 + r) * 3 + cc) * 130
                            dst = vrr[:, off:off + 130].rearrange(
                                "p (two w) -> p two w", two=2)[:, :, 0:64]
                            nc.vector.tensor_copy(
                                dst, pvr.rearrange("p (two w) -> p two w", two=2))

                    # ---------- attention + output proj for chunk cc ----------
                    c = cc
                    qci = ci
                    ot0 = pot.tile([65, 512], F32, tag="ot", name=f"ot0_{qci}")
                    ot1 = pot.tile([65, 512], F32, tag="ot", name=f"ot1_{qci}")
                    ots = (ot0, ot1)
                    ot4 = [o.rearrange("p (i r) -> p r i", r=4) for o in ots]
                    jcap = 128 * (4 * c - 2) if COMPACT else 0
                    jb_lo = max(0, 4 * c - 2) if COMPACT else 0
                    qt4 = [qts[qci][h * 64:(h + 1) * 64, :].rearrange(
                        "p (i r) -> p r i", r=4) for h in (0, 1)]
                    kt4 = [ktb[b][h * 64:(h + 1) * 64, :].rearrange(
                        "p (j r) -> p r j", r=4) for h in (0, 1)]

                    # ---- window / mixed stream ----
                    for jb in range(jb_lo, 4 * c + 4):
                        g = b * JB + jb
                        im0 = max(0, 128 * jb - 512 * c) if TRIM else 0
                        scs = []
                        for h in (0, 1):
                            sch = psc.tile([128, 512], F32, tag="sc",
                                           name=f"sc{qci}_{jb}_{h}")
                            nc.tensor.matmul(
                                sch[:, im0:512],
                                ktb[b][h * 64:(h + 1) * 64, jb * 128:(jb + 1) * 128],
                                qts[qci][h * 64:(h + 1) * 64, im0:512],
                                start=True, stop=True)
                            scs.append(sch)
                        e2 = epool.tile([128, 2, 512], BF16, tag="e", name=f"e{qci}_{jb}")
                        for h in (0, 1):
                            nc.scalar.activation(e2[:, h, im0:512], scs[h][:, im0:512],
                                                 mybir.ActivationFunctionType.Exp)
                        em2 = epool.tile([128, 2, 512], BF16, tag="em", name=f"em{qci}_{jb}")
                        t0 = T0 + 512 * c - 128 * jb
                        wsl = w01_sb[:, t0 + im0:t0 + 512]
                        wb = bass.AP(tensor=wsl.tensor, offset=wsl.offset,
                                     ap=[list(wsl.ap[0]), [0, 2], list(wsl.ap[1])])
                        nc.vector.tensor_mul(em2[:, :, im0:512], e2[:, :, im0:512], wb)
                        for h in (0, 1):
                            nc.tensor.matmul(
                                ots[h][:, im0:512],
                                vhh[:, g * 130 + h * 65:g * 130 + h * 65 + 65],
                                em2[:, h, im0:512],
                                start=(jb == jb_lo), stop=False, skip_group_check=True)

                    # ---- deep-dilated compact stream (all-valid; maskless) ----
                    jmtot = max(jcap, 0) // 4
                    njmb = (jmtot + 127) // 128
                    for jmb in range(njmb):
                        jm0 = 128 * jmb
                        jmw = min(128, jmtot - jm0)
                        scds = [psc.tile([128, 512], F32, tag="sc",
                                         name=f"scd{qci}_{jmb}_{h}") for h in (0, 1)]
                        for r in range(4):
                            for h in (0, 1):
                                nc.tensor.matmul(
                                    scds[h][0:jmw, r * 128:(r + 1) * 128],
                                    kt4[h][:, r, jm0:jm0 + jmw],
                                    qt4[h][:, r, :],
                                    start=True, stop=True, skip_group_check=True)
                        ed2 = epool.tile([128, 2, 512], BF16, tag="e", name=f"ed{qci}_{jmb}")
                        for h in (0, 1):
                            nc.scalar.activation(ed2[0:jmw, h, :], scds[h][0:jmw, :],
                                                 mybir.ActivationFunctionType.Exp)
                        last = (jmb == njmb - 1)
                        for r in range(4):
                            for h in (0, 1):
                                off = ((b * 4 + r) * 3 + jmb) * 130 + h * 65
                                nc.tensor.matmul(
                                    ot4[h][:, r, :],
                                    vrr[0:jmw, off:off + 65],
                                    ed2[0:jmw, h, r * 128:(r + 1) * 128],
                                    start=False, stop=(last and r == 3),
                                    skip_group_check=True)

                    # softmax normalize:  O.T = O_aug.T[0:64] / O_aug.T[64]
                    ot2 = opool.tile([128, 512], BF16, tag="ot2", name=f"ot2_{qci}")
                    rc0 = spool.tile([1, 512], F32, tag="rc", name=f"rc0_{qci}")
                    rc1 = spool.tile([1, 512], F32, tag="rc", name=f"rc1_{qci}")
                    nc.vector.reciprocal(rc0, ot0[64:65, :])
                    nc.vector.reciprocal(rc1, ot1[64:65, :])
                    rb0 = spool.tile([64, 512], F32, tag="rb", name=f"rb0_{qci}")
                    rb1 = spool.tile([64, 512], F32, tag="rb", name=f"rb1_{qci}")
                    nc.gpsimd.partition_broadcast(rb0, rc0)
                    nc.gpsimd.partition_broadcast(rb1, rc1)
                    nc.vector.tensor_mul(ot2[0:64, :], ot0[0:64, :], rb0)
                    t1 = spool.tile([64, 512], BF16, tag="t1", name=f"t1_{qci}")
                    nc.vector.tensor_mul(t1, ot1[0:64, :], rb1)
                    # cross-partition move of head1 rows to partitions 64..127
                    nc.sync.dma_start(ot2[64:128, :], t1)

                    # output projection for this i-chunk (psum shares "sc" slots)
                    ysb2 = ypool.tile([128, 4, D], BF16, tag="ysb", name=f"ysb{qci}")
                    for ib in range(4):
                        for oc in range(2):
                            py = psc.tile([128, 512], F32, tag="sc",
                                          name=f"py{qci}_{ib}_{oc}")
                            nc.tensor.matmul(py, ot2[:, ib * 128:(ib + 1) * 128],
                                             wo_sb[:, oc * 512:(oc + 1) * 512],
                                             start=True, stop=True)
                            dst = ysb2[:, ib, oc * 512:(oc + 1) * 512]
                            if (ib * 2 + oc) % 2 == 0:
                                nc.scalar.copy(dst, py)
                            else:
                                nc.vector.tensor_copy(dst, py)
                    row0 = b * S + c * 512
                    yv = y[row0:row0 + 512, :].rearrange("(i p) o -> p i o", p=128)
                    nc.sync.dma_start(yv, ysb2)


_NC_CACHE = None


def _get_nc():
    global _NC_CACHE
    if _NC_CACHE is None:
        nc = bacc.Bacc("TRN2", target_bir_lowering=False, debug=False,
                       num_devices=NCORES)
        with tile.TileContext(nc) as tc:
            build_tile_kernel(tc)
        nc.compile()
        _NC_CACHE = nc
    return _NC_CACHE


def _mask_band():
    """W01[j', t] = 1 if delta = t - T0 - j' is an allowed attention offset."""
    jj = np.arange(128)[:, None]
    tt = np.arange(TW)[None, :]
    delta = tt - T0 - jj
    win = (delta >= 0) & (delta <= WINDOW - 1)
    dil = (delta >= WINDOW + DIL) & (delta % DIL == 0)
    return (win | dil).astype(NPBF16)


def make_in_maps(x, qkv, wo):
    # xT2[p, ft, n] = x[n, ft*128 + p]
    xT2 = np.ascontiguousarray(x.reshape(N, 8, 128).transpose(2, 1, 0)).astype(NPBF16)
    w01 = _mask_band()
    in_maps = []
    for d in range(NCORES):
        r0 = d * DH2
        wq = qkv[r0:r0 + DH2, :] * np.float32(1.0 / np.sqrt(DH))
        wk = qkv[D + r0:D + r0 + DH2, :]
        wv = qkv[2 * D + r0:2 * D + r0 + DH2, :]
        cat = np.concatenate([wq.T, wk.T, wv.T], axis=1)  # (D, 3*DH2)
        wqkv = np.ascontiguousarray(
            cat.reshape(8, 128, 3 * DH2).transpose(1, 0, 2)).astype(NPBF16)
        in_maps.append({
            "xT": xT2,
            "wqkvT": wqkv,
            "woT": np.ascontiguousarray(wo[:, r0:r0 + DH2].T).astype(NPBF16),
            "w01": w01,
        })
    return in_maps


def run(x, qkv, wo, trace=False):
    nc = _get_nc()
    in_maps = make_in_maps(x, qkv, wo)
    try:
        res = run_bass_kernel_spmd(nc, in_maps, core_ids=list(range(NCORES)),
                                   trace=trace)
    except ModuleNotFoundError:
        # NTFF profiling hook unavailable in this environment
        res = run_bass_kernel_spmd(nc, in_maps, core_ids=list(range(NCORES)),
                                   trace=False)
    acc = None
    for r in res.results:
        part = np.asarray(r["y"], dtype=np.float32)
        acc = part if acc is None else acc + part
    out = acc.reshape(B, S, D).astype(np.float32)
    return out, res


def kernel(x, qkv, wo):
    out, _ = run(np.asarray(x, dtype=np.float32),
                 np.asarray(qkv, dtype=np.float32),
                 np.asarray(wo, dtype=np.float32))
    return out


# revision 33
# speedup vs baseline: 1.1508x; 1.1508x over previous
"""Trainium2 Bass kernel for CustomWindowMHA (sparse window+dilated attention).

Problem (hardcoded):
  x: (2, 2048, 1024) f32, qkv: (3072, 1024) f32, wo: (1024, 1024) f32
  H=16 heads, dh=64, window=128, dilation=4.
  out = softmax(mask(QK^T/8)) V @ wo^T          (B, S, D) f32

Sharding: 16 heads / 8 cores = 2 heads per core (head-parallel).  Each core
computes its 2 heads' attention output O_d (n=4096, 128) and the partial
Y_d = O_d @ wo[:, d-slice]^T (4096, 1024).  Host sums the 8 partials.

Per-core layout (everything chained transposed so activations never need a
transpose; compute dtype bf16 with fp32 PSUM accumulation):
  xT (1024, 4096) bf16 [f, n]   host-pretransposed, replicated
  QT = (wq_d/8) @ xT : (128, 4096)  [2*dh, n]   (1/8 folded into weights)
  KT =  wk_d    @ xT : (128, 4096)
  VT =  wv_d    @ xT : (128, 4096) -> PE-transpose -> V natural (j, dh) bf16
  Attention is split into two streams that accumulate into one PSUM tile:
  - window/mixed stream (j-blocks near the diagonal): scores_T per head,
    exp (ACT), multiplicative 0/1 W01[delta] mask (DVE), AV with [V|1]
    aug so row 64 carries sum(exp); leading all-invalid columns trimmed.
  - deep-dilated stream (j < 128*(4c-2), where delta >= 257 everywhere):
    residue-compacted:  for r = i%4, S_r[jm, im] = K[4jm+r] . Q[4im+r];
    every computed entry is valid -> NO mask work at all.  AV uses
    stride-4-gathered V_r and writes strided PSUM columns i%4==r of the
    same O_aug.T accumulator.
  O.T = O_aug.T[0:64] * (1/row64)  (DVE recip + gpsimd partition_broadcast)
  Y tile = O2.T-slice^T . woT -> SBUF -> DMA   (f32 out, host sums partials)
"""

import os
import sys

import numpy as np

for _p in ("/opt/trn_rl_repo",):
    if _p not in sys.path and os.path.isdir(_p):
        sys.path.append(_p)

import ml_dtypes
import concourse.bacc as bacc
import concourse.bass as bass
import concourse.mybir as mybir
import concourse.tile as tile
from concourse.bass_utils import run_bass_kernel_spmd
from concourse.masks import make_identity

F32 = mybir.dt.float32
BF16 = mybir.dt.bfloat16
NPBF16 = ml_dtypes.bfloat16

B, S, D = 2, 2048, 1024
H, DH = 16, 64
WINDOW, DIL = 128, 4
NCORES = 8
N = B * S          # 4096 rows total
DH2 = 2 * DH       # 128 dims per core (2 heads)
NCH = N // 512     # 8 n-chunks of 512
CPB = S // 512     # 4 i-chunks per batch
JB = S // 128      # 16 j-blocks per batch
T0 = 384           # W01 band offset: delta = t - T0 - j'
TW = T0 + 512 * (CPB - 1) + 512  # 2432 band width
COMPACT = os.environ.get("K_COMPACT", "1") == "1"
TRIM = os.environ.get("K_TRIM", "1") == "1"


def build_tile_kernel(tc):
    nc = tc.nc

    xT = nc.dram_tensor("xT", [128, 8, N], BF16, kind="ExternalInput").ap()
    wqkvT = nc.dram_tensor("wqkvT", [128, 8, 3 * DH2], BF16, kind="ExternalInput").ap()
    woT = nc.dram_tensor("woT", [DH2, D], BF16, kind="ExternalInput").ap()
    w01 = nc.dram_tensor("w01", [128, TW], BF16, kind="ExternalInput").ap()
    w02 = nc.dram_tensor("w02", [128, 128], BF16, kind="ExternalInput").ap()
    y = nc.dram_tensor("y", [N, D], BF16, kind="ExternalOutput").ap()

    with (
        tc.tile_pool(name="const", bufs=1) as cpool,
        tc.tile_pool(name="xin", bufs=3) as xpool,
        tc.tile_pool(name="qk", bufs=8) as qkpool,
        tc.tile_pool(name="vtp", bufs=2) as vtpool,
        tc.tile_pool(name="pers", bufs=1) as perspool,
        tc.tile_pool(name="exp", bufs=4) as epool,
        tc.tile_pool(name="ot2", bufs=4) as opool,
        tc.tile_pool(name="small", bufs=4) as spool,
        tc.tile_pool(name="yst", bufs=2) as ypool,
    ):
        # ---- constants / weights in SBUF ----
        wqkv_sb = cpool.tile([128, 8, 3 * DH2], BF16, name="wqkv_sb")
        nc.sync.dma_start(wqkv_sb[:, 0, :], wqkvT[:, 0, :])
        wo_sb = cpool.tile([128, D], BF16, name="wo_sb")
        w01_sb = cpool.tile([128, TW], BF16, name="w01_sb")
        w02_sb = cpool.tile([128, 128], BF16, name="w02_sb")
        ident = cpool.tile([128, 128], BF16, name="ident")
        make_identity(nc, ident)

        # per-head V with ones column, interleaved: per j-block g a 130-col
        # group [V_h0(64) | 1 | V_h1(64) | 1]
        vhh = perspool.tile([128, 2 * JB * 130], BF16, name="vhh")
        nc.vector.memset(
            vhh.rearrange("p (g w) -> p g w", w=65)[:, :, 64:65], 1.0)
        # residue-gathered V for the compact stream: (b, r, jmb<3) 130-groups
        vrr = perspool.tile([128, B * 4 * 3 * 130], BF16, name="vrr")
        nc.vector.memset(
            vrr.rearrange("p (g w) -> p g w", w=65)[:, :, 64:65], 1.0)

        qts = [None] * NCH
        ktb = [None] * B
        vtb = [None] * B

        for bb in range(B):
            with (
                tc.tile_pool(name=f"pp1_{bb}", bufs=2, space="PSUM") as pp1,
                tc.tile_pool(name=f"psc_{bb}", bufs=4, space="PSUM") as psc,
                tc.tile_pool(name=f"pot_{bb}", bufs=2, space="PSUM") as pot,
            ):
                b = bb
                kt = qkpool.tile([128, S], BF16, tag="kt", name=f"ktb{bb}", bufs=2)
                vt = vtpool.tile([128, S], BF16, tag="vt", name=f"vtb{bb}", bufs=2)
                ktb[bb] = kt
                vtb[bb] = vt
                vt4 = vt.rearrange("p (j r) -> p r j", r=4)
                for cc in range(CPB):
                    # ---------- projections for chunk cc ----------
                    ci = bb * CPB + cc
                    n0 = ci * 512
                    xtb = xpool.tile([128, 8, 512], BF16, tag="xt", name=f"xt{ci}")
                    if ci == 0:
                        # first f-tile alone so the very first matmul can start
                        nc.sync.dma_start(xtb[:, 0, :], xT[:, 0, n0:n0 + 512])
                        nc.sync.dma_start(wqkv_sb[:, 1:8, :], wqkvT[:, 1:8, :])
                        nc.sync.dma_start(xtb[:, 1:8, :], xT[:, 1:8, n0:n0 + 512])
                        nc.sync.dma_start(wo_sb, woT)
                        nc.sync.dma_start(w01_sb, w01)
                        nc.sync.dma_start(w02_sb, w02)
                    else:
                        nc.sync.dma_start(xtb, xT[:, :, n0:n0 + 512])
                    xts = [xtb[:, ft, :] for ft in range(8)]

                    psq = pp1.tile([128, 512], F32, tag="proj", name=f"psq{ci}")
                    for ft in range(8):
                        nc.tensor.matmul(psq, wqkv_sb[:, ft, 0:DH2], xts[ft],
                                         start=(ft == 0), stop=(ft == 7))
                    qt = qkpool.tile([128, 512], BF16, tag="qt", name=f"qt{ci}")
                    nc.scalar.copy(qt, psq)
                    qts[ci] = qt

                    psk = pp1.tile([128, 512], F32, tag="proj", name=f"psk{ci}")
                    for ft in range(8):
                        nc.tensor.matmul(psk, wqkv_sb[:, ft, DH2:2 * DH2], xts[ft],
                                         start=(ft == 0), stop=(ft == 7))
                    nc.scalar.copy(kt[:, cc * 512:cc * 512 + 512], psk)

                    psv = pp1.tile([128, 512], F32, tag="proj", name=f"psv{ci}")
                    for ft in range(8):
                        nc.tensor.matmul(psv, wqkv_sb[:, ft, 2 * DH2:3 * DH2], xts[ft],
                                         start=(ft == 0), stop=(ft == 7))
                    nc.scalar.copy(vt[:, cc * 512:cc * 512 + 512], psv)

                    # natural V blocks for this chunk
                    for sub in range(4):
                        g = ci * 4 + sub
                        pvt = pp1.tile([128, 128], BF16, tag="proj", name=f"pvt{g}")
                        nc.tensor.transpose(
                            pvt, vt[:, cc * 512 + sub * 128:cc * 512 + (sub + 1) * 128],
                            ident)
                        dst = vhh[:, g * 130:g * 130 + 130].rearrange(
                            "p (two w) -> p two w", two=2)[:, :, 0:64]
                        nc.vector.tensor_copy(
                            dst, pvt.rearrange("p (two w) -> p two w", two=2))
                    # residue-gathered V_r block cc (needed by chunks > cc)
                    if cc < 3:
                        for r in range(4):
                            pvr = pp1.tile([128, 128], BF16, tag="proj",
                                            name=f"pvr{bb}_{r}_{cc}")
                            nc.tensor.transpose(
                                pvr, vt4[:, r, cc * 128:(cc + 1) * 128], ident)
                            off = ((bb * 4 + r) * 3 + cc) * 130
                            dst = vrr[:, off:off + 130].rearrange(
                                "p (two w) -> p two w", two=2)[:, :, 0:64]
                            nc.vector.tensor_copy(
                                dst, pvr.rearrange("p (two w) -> p two w", two=2))

                    # ---------- attention + output proj for chunk cc ----------
                    c = cc
                    qci = ci
                    ot0 = pot.tile([65, 512], F32, tag="ot", name=f"ot0_{qci}")
                    ot1 = pot.tile([65, 512], F32, tag="ot", name=f"ot1_{qci}")
                    ots = (ot0, ot1)
                    ot4 = [o.rearrange("p (i r) -> p r i", r=4) for o in ots]
                    jcap = 128 * (4 * c - 1) if COMPACT else 0
                    jb_lo = max(0, 4 * c - 1) if COMPACT else 0
                    qt4 = [qts[qci][h * 64:(h + 1) * 64, :].rearrange(
                        "p (i r) -> p r i", r=4) for h in (0, 1)]
                    kt4 = [ktb[b][h * 64:(h + 1) * 64, :].rearrange(
                        "p (j r) -> p r j", r=4) for h in (0, 1)]

                    # ---- window / mixed stream ----
                    for jb in range(jb_lo, 4 * c + 4):
                        g = b * JB + jb
                        im0 = max(0, 128 * jb - 512 * c) if TRIM else 0
                        scs = []
                        for h in (0, 1):
                            sch = psc.tile([128, 512], F32, tag="sc",
                                           name=f"sc{qci}_{jb}_{h}")
                            nc.tensor.matmul(
                                sch[:, im0:512],
                                ktb[b][h * 64:(h + 1) * 64, jb * 128:(jb + 1) * 128],
                                qts[qci][h * 64:(h + 1) * 64, im0:512],
                                start=True, stop=True)
                            scs.append(sch)
                        e2 = epool.tile([128, 2, 512], BF16, tag="e", name=f"e{qci}_{jb}")
                        for h in (0, 1):
                            nc.scalar.activation(e2[:, h, im0:512], scs[h][:, im0:512],
                                                 mybir.ActivationFunctionType.Exp)
                        em2 = epool.tile([128, 2, 512], BF16, tag="em", name=f"em{qci}_{jb}")
                        t0 = T0 + 512 * c - 128 * jb
                        wsl = w01_sb[:, t0 + im0:t0 + 512]
                        wb = bass.AP(tensor=wsl.tensor, offset=wsl.offset,
                                     ap=[list(wsl.ap[0]), [0, 2], list(wsl.ap[1])])
                        nc.vector.tensor_mul(em2[:, :, im0:512], e2[:, :, im0:512], wb)
                        for h in (0, 1):
                            nc.tensor.matmul(
                                ots[h][:, im0:512],
                                vhh[:, g * 130 + h * 65:g * 130 + h * 65 + 65],
                                em2[:, h, im0:512],
                                start=(jb == jb_lo), stop=False, skip_group_check=True)

                    # ---- deep-dilated compact stream (all-valid; maskless) ----
                    jmtot = max(jcap, 0) // 4
                    njmb = (jmtot + 127) // 128
                    for jmb in range(njmb):
                        jm0 = 128 * jmb
                        jmw = min(128, jmtot - jm0)
                        scds = [psc.tile([128, 512], F32, tag="sc",
                                         name=f"scd{qci}_{jmb}_{h}") for h in (0, 1)]
                        for r in range(4):
                            for h in (0, 1):
                                nc.tensor.matmul(
                                    scds[h][0:jmw, r * 128:(r + 1) * 128],
                                    kt4[h][:, r, jm0:jm0 + jmw],
                                    qt4[h][:, r, :],
                                    start=True, stop=True, skip_group_check=True)
                        ed2 = epool.tile([128, 2, 512], BF16, tag="e", name=f"ed{qci}_{jmb}")
                        for h in (0, 1):
                            nc.scalar.activation(ed2[0:jmw, h, :], scds[h][0:jmw, :],
                                                 mybir.ActivationFunctionType.Exp)
                        last = (jmb == njmb - 1)
                        for r in range(4):
                            for h in (0, 1):
                                off = ((b * 4 + r) * 3 + jmb) * 130 + h * 65
                                nc.tensor.matmul(
                                    ot4[h][:, r, :],
                                    vrr[0:jmw, off:off + 65],
                                    ed2[0:jmw, h, r * 128:(r + 1) * 128],
                                    start=False, stop=(last and r == 3),
                                    skip_group_check=True)

                    # softmax normalize:  O.T = O_aug.T[0:64] / O_aug.T[64]
                    ot2 = opool.tile([128, 512], BF16, tag="ot2", name=f"ot2_{qci}")
                    rc0 = spool.tile([1, 512], F32, tag="rc", name=f"rc0_{qci}")
                    rc1 = spool.tile([1, 512], F32, tag="rc", name=f"rc1_{qci}")
                    nc.vector.reciprocal(rc0, ot0[64:65, :])
                    nc.vector.reciprocal(rc1, ot1[64:65, :])
                    rb0 = spool.tile([64, 512], F32, tag="rb", name=f"rb0_{qci}")
                    rb1 = spool.tile([64, 512], F32, tag="rb", name=f"rb1_{qci}")
                    nc.gpsimd.partition_broadcast(rb0, rc0)
                    nc.gpsimd.partition_broadcast(rb1, rc1)
                    nc.vector.tensor_mul(ot2[0:64, :], ot0[0:64, :], rb0)
                    t1 = spool.tile([64, 512], BF16, tag="t1", name=f"t1_{qci}")
                    nc.vector.tensor_mul(t1, ot1[0:64, :], rb1)
                    # cross-partition move of head1 rows to partitions 64..127
                    nc.sync.dma_start(ot2[64:128, :], t1)

                    # output projection for this i-chunk (psum shares "sc" slots)
                    ysb2 = ypool.tile([128, 4, D], BF16, tag="ysb", name=f"ysb{qci}")
                    for ib in range(4):
                        for oc in range(2):
                            py = psc.tile([128, 512], F32, tag="sc",
                                          name=f"py{qci}_{ib}_{oc}")
                            nc.tensor.matmul(py, ot2[:, ib * 128:(ib + 1) * 128],
                                             wo_sb[:, oc * 512:(oc + 1) * 512],
                                             start=True, stop=True)
                            dst = ysb2[:, ib, oc * 512:(oc + 1) * 512]
                            if (ib * 2 + oc) % 2 == 0:
                                nc.scalar.copy(dst, py)
                            else:
                                nc.vector.tensor_copy(dst, py)
                    row0 = b * S + c * 512
                    yv = y[row0:row0 + 512, :].rearrange("(i p) o -> p i o", p=128)
                    nc.sync.dma_start(yv[:, 0:2, :], ysb2[:, 0:2, :])
                    nc.sync.dma_start(yv[:, 2:4, :], ysb2[:, 2:4, :])


_NC_CACHE = None


def _get_nc():
    global _NC_CACHE
    if _NC_CACHE is None:
        nc = bacc.Bacc("TRN2", target_bir_lowering=False, debug=False,
                       num_devices=NCORES)
        with tile.TileContext(nc) as tc:
            build_tile_kernel(tc)
        nc.compile()
        _NC_CACHE = nc
    return _NC_CACHE


def _mask_band():
    """W01[j', t] = 1 if delta = t - T0 - j' is an allowed attention offset."""
    jj = np.arange(128)[:, None]
    tt = np.arange(TW)[None, :]
    delta = tt - T0 - jj
    win = (delta >= 0) & (delta <= WINDOW - 1)
    dil = (delta >= WINDOW + DIL) & (delta % DIL == 0)
    return (win | dil).astype(NPBF16)


def _mask_top_block():
    """W2[jm', i'] = 1 iff (i' - jm' + 128) >= 33, i.e. delta >= 132 in the
    top residue-compacted jm-block."""
    jj = np.arange(128)[:, None]
    ii = np.arange(128)[None, :]
    return ((ii - jj + 128) >= 33).astype(NPBF16)


def make_in_maps(x, qkv, wo):
    # xT2[p, ft, n] = x[n, ft*128 + p]
    xT2 = np.ascontiguousarray(x.reshape(N, 8, 128).transpose(2, 1, 0)).astype(NPBF16)
    w01 = _mask_band()
    in_maps = []
    for d in range(NCORES):
        r0 = d * DH2
        wq = qkv[r0:r0 + DH2, :] * np.float32(1.0 / np.sqrt(DH))
        wk = qkv[D + r0:D + r0 + DH2, :]
        wv = qkv[2 * D + r0:2 * D + r0 + DH2, :]
        cat = np.concatenate([wq.T, wk.T, wv.T], axis=1)  # (D, 3*DH2)
        wqkv = np.ascontiguousarray(
            cat.reshape(8, 128, 3 * DH2).transpose(1, 0, 2)).astype(NPBF16)
        in_maps.append({
            "xT": xT2,
            "wqkvT": wqkv,
            "woT": np.ascontiguousarray(wo[:, r0:r0 + DH2].T).astype(NPBF16),
            "w01": w01,
            "w02": _mask_top_block(),
        })
    return in_maps


def run(x, qkv, wo, trace=False):
    nc = _get_nc()
    in_maps = make_in_maps(x, qkv, wo)
    try:
        res = run_bass_kernel_spmd(nc, in_maps, core_ids=list(range(NCORES)),
                                   trace=trace)
    except ModuleNotFoundError:
        # NTFF profiling hook unavailable in this environment
        res = run_bass_kernel_spmd(nc, in_maps, core_ids=list(range(NCORES)),
                                   trace=False)
    acc = None
    for r in res.results:
        part = np.asarray(r["y"], dtype=np.float32)
        acc = part if acc is None else acc + part
    out = acc.reshape(B, S, D).astype(np.float32)
    return out, res


def kernel(x, qkv, wo):
    out, _ = run(np.asarray(x, dtype=np.float32),
                 np.asarray(qkv, dtype=np.float32),
                 np.asarray(wo, dtype=np.float32))
    return out
